# revision 29
# baseline (speedup 1.0000x reference)
"""Single-head causal attention (B=4, S=4096, d_model=512, d_head=64) on 8 trn2 cores.

Sharding: core c handles batch c%4 and the key-chunk parity p=c//4 — it sees
ALL 4096 queries of its batch but only the 16 key-chunks (of 128 keys) with
global index % 2 == p.  Each core produces the unnormalized partial
attention O'^T = V'^T P^T (with a ones-column giving per-query partial
softmax denominators in row 64); the host adds the two partials per batch
and normalizes.  This is exactly softmax split over keys, and it makes the
program perfectly uniform: every core runs slots g=0..7 (512 queries each)
with exactly 2g+2 local key chunks — no padding, no garbage work.

Causality: local chunk j of slot g covers global keys 128*(2j+p)+x; chunks
j <= 2g-1 are fully attended, and only the last two chunks (j=2g, 2g+1) are
masked.  The mask threshold 128*(2t+p)+x is slot-independent, so a single
[128, 1024] mask (computed on-chip from a 2-column threshold table) covers
every slot's final pair.

Device algorithm (bf16 data, fp32 accumulation):
  QT = Wq^T qT  [64, 4096]  (PE, psum->sbuf copies on DVE)
  KT = Wk^T kT  [64, 2048]
  V  = (vT chunks)^T Wv row-form [128, 16, 64] + ones column -> V' [128,16,65]
  per slot g, chunk pair i (software-pipelined one pair ahead):
     S^T[:, 0:512]   = K_{2i} Q_g^T   (PSUM [128, 1024] two banks)
     S^T[:, 512:1024]= K_{2i+1} Q_g^T
     P^T = exp(S^T/8)  (one ACT instruction over [128, 1024] -> bf16)
     if last pair: P^T *= mask (DVE, 4x mode)
     O'[65, 512] += V'_{2i}^T P^T_lo + V'_{2i+1}^T P^T_hi  (PE)
  O' slots DMA'd straight from PSUM to DRAM; host combines + normalizes.
"""

import os
import sys

import numpy as np

for _p in ("/opt/trn_rl_repo",):
    if _p not in sys.path and os.path.isdir(_p):
        sys.path.insert(0, _p)

import ml_dtypes

B, S, DM, DH = 4, 4096, 512, 64
NCORES = 8
NSLOTS = 8                    # query slots of 512 rows (full batch per core)
QG = 512
NKC_LOC = 16                  # local key chunks of 128 keys per core

BF16 = ml_dtypes.bfloat16

_PROGRAM = None


def _build_program():
    import concourse.bacc as bacc
    import concourse.mybir as mybir
    import concourse.tile as tile

    f32 = mybir.dt.float32
    bf16 = mybir.dt.bfloat16
    fp8 = mybir.dt.float8e4
    Exp = mybir.ActivationFunctionType.Exp
    DoubleRow = mybir.MatmulPerfMode.DoubleRow

    nc = bacc.Bacc("TRN2", target_bir_lowering=False, debug=False,
                   num_devices=NCORES)

    qT = nc.dram_tensor("qT", [DM, S], bf16, kind="ExternalInput").ap()
    kT = nc.dram_tensor("kT", [DM, S // 2], bf16, kind="ExternalInput").ap()
    vT = nc.dram_tensor("vT", [DM, S // 2], bf16, kind="ExternalInput").ap()
    # wq | wk | wv concatenated, pre-arranged on host to the SBUF layout
    # [128, (dm_chunk, 3*dh)] so the DMA has >=512B contiguous runs
    w3 = nc.dram_tensor("w3", [128, 4 * 3 * DH], bf16, kind="ExternalInput").ap()
    # thr[x, t] = 128*(2t + parity) + x : q-column threshold for the two
    # masked chunks of every slot's final pair
    thr = nc.dram_tensor("thr", [128, 2], f32, kind="ExternalInput").ap()
    # out[d, g, q] : rows 0..63 = unnormalized O'^T, row 64 = partial denom
    out = nc.dram_tensor("out", [DH + 1, NSLOTS, QG], f32,
                         kind="ExternalOutput").ap()

    qT_r = qT.rearrange("(c p) n -> p c n", p=128)
    kT_r = kT.rearrange("(c p) n -> p c n", p=128)
    vT_r = vT.rearrange("(c p) n -> p c n", p=128)

    with tile.TileContext(nc) as tc:
        with (
            tc.tile_pool(name="consts", bufs=1) as consts,
            tc.tile_pool(name="big", bufs=1) as big,
            tc.tile_pool(name="work", bufs=3) as work,
            tc.tile_pool(name="pst", bufs=2, space="PSUM") as pst,
            tc.tile_pool(name="pacc", bufs=2, space="PSUM") as pacc,
            tc.tile_pool(name="psm", bufs=2, space="PSUM") as psm,
        ):
            # ---- constants ----
            w_sb = consts.tile([128, 4, 3 * DH], bf16)
            nc.sync.dma_start(out=w_sb,
                              in_=w3.rearrange("p (c d) -> p c d", c=4))

            def wq_sb(c):
                return w_sb[:, c, 0:DH]

            def wk_sb(c):
                return w_sb[:, c, DH:2 * DH]

            def wv_sb(c):
                return w_sb[:, c, 2 * DH:3 * DH]

            thr_sb = consts.tile([128, 2], f32)
            yiota = consts.tile([128, QG], f32)
            masks = consts.tile([128, 2 * QG], bf16)

            def gen_masks():
                # on gpsimd: keeps the DVE queue free for projection copies.
                # Emitted after the fp8 zero memsets (Pool queue is in-order
                # and the is_ge waits on the thr DMA).
                nc.gpsimd.iota(yiota, pattern=[[1, QG]], base=0,
                               channel_multiplier=0,
                               allow_small_or_imprecise_dtypes=True)
                for t in range(2):
                    nc.gpsimd.tensor_scalar(
                        out=masks[:, t * QG:(t + 1) * QG], in0=yiota,
                        scalar1=thr_sb[:, t:t + 1], scalar2=None,
                        op0=mybir.AluOpType.is_ge)

            # ---- staged inputs / projected tensors ----
            qT_sb = big.tile([128, 4, S], bf16)
            kT_sb = big.tile([128, 4, S // 2], bf16)
            vT_sb = big.tile([128, 4, S // 2], bf16)
            # Q/K projected then quantized to fp8 for DoubleRow score matmuls;
            # layout [64, 2, n]: half 0 = real 64-dim values, half 1 = zeros
            # (DoubleRow contracts over both halves; the zero half is free and
            # lets the fp8 conversion be a single 64-partition copy)
            q8_sb = big.tile([64, 2, S], fp8)
            k8_sb = big.tile([64, 2, S // 2], fp8)
            # zero halves in pieces, first-needed first, so early score
            # matmuls aren't gated on one huge memset
            nc.gpsimd.memset(k8_sb[:, 1, 0:QG], 0.0)
            nc.gpsimd.memset(q8_sb[:, 1, 0:2 * QG], 0.0)
            gen_masks()
            for kp in range(1, 4):
                nc.gpsimd.memset(k8_sb[:, 1, kp * QG:(kp + 1) * QG], 0.0)
            for gp in range(1, 4):
                nc.gpsimd.memset(
                    q8_sb[:, 1, 2 * gp * QG:2 * (gp + 1) * QG], 0.0)
            v_sb = big.tile([128, NKC_LOC, DH + 1], bf16)
            nc.vector.memset(v_sb[:, :, DH:DH + 1], 1.0)
            out_sb = big.tile([DH + 1, NSLOTS, QG], f32)

            def dma_q(g):
                nc.sync.dma_start(out=qT_sb[:, :, g * QG:(g + 1) * QG],
                                  in_=qT_r[:, :, g * QG:(g + 1) * QG])

            def dma_k(c0, n):
                sl = slice(c0, c0 + n)
                nc.sync.dma_start(out=kT_sb[:, :, sl], in_=kT_r[:, :, sl])

            def dma_v(c0, n):
                sl = slice(c0, c0 + n)
                nc.sync.dma_start(out=vT_sb[:, :, sl], in_=vT_r[:, :, sl])

            def proj_q(g):
                sl = slice(g * QG, (g + 1) * QG)
                pp = psm.tile([64, QG], f32, tag="sm", name="pq")
                for c in range(4):
                    nc.tensor.matmul(pp, lhsT=wq_sb(c),
                                     rhs=qT_sb[:, c, sl],
                                     start=(c == 0), stop=(c == 3))
                nc.vector.tensor_copy(q8_sb[:, 0, sl], pp)

            def proj_k(col0, ncols, on_act=False):
                sl = slice(col0, col0 + ncols)
                pp = psm.tile([64, QG], f32, tag="sm", name="pk")
                for c in range(4):
                    nc.tensor.matmul(pp[:, 0:ncols], lhsT=wk_sb(c),
                                     rhs=kT_sb[:, c, sl],
                                     start=(c == 0), stop=(c == 3))
                if on_act:  # head only: ACT is idle before the first exp
                    nc.scalar.copy(k8_sb[:, 0, sl], pp[:, 0:ncols])
                else:
                    nc.vector.tensor_copy(k8_sb[:, 0, sl], pp[:, 0:ncols])

            def proj_v(kb):
                pv = psm.tile([128, DH], f32, tag="sm", name="pv")
                for c in range(4):
                    nc.tensor.matmul(pv,
                                     lhsT=vT_sb[:, c, kb * 128:(kb + 1) * 128],
                                     rhs=wv_sb(c),
                                     start=(c == 0), stop=(c == 3))
                nc.vector.tensor_copy(v_sb[:, kb, 0:DH], pv)

            # ---- software-pipelined main sequence ----
            # pairs[n] = (slot g, pair index i); stage A (scores) is emitted
            # one pair ahead of stage EP (exp + PV accumulate) so the PE has
            # score work queued while ACT runs the previous pair's exp.
            pairs = [(g, i) for g in range(NSLOTS) for i in range(g + 1)]
            npairs = len(pairs)

            sts = {}
            ots = {}

            def stage_A(n):
                g, i = pairs[n]
                st = pst.tile([128, 2 * QG], f32, tag="st", name="st")
                sts[n] = st
                qcols = q8_sb[:, :, g * QG:(g + 1) * QG]
                for h in range(2):
                    j = 2 * i + h
                    nc.tensor.matmul(st[:, h * QG:(h + 1) * QG],
                                     lhsT=k8_sb[:, :, j * 128:(j + 1) * 128],
                                     rhs=qcols, start=True, stop=True,
                                     perf_mode=DoubleRow)

            def stage_EP(n):
                g, i = pairs[n]
                st = sts.pop(n)
                pt = work.tile([128, 2 * QG], bf16, tag="pt", name="pt")
                nc.scalar.activation(pt, st, Exp, scale=0.125)
                if i == g:
                    nc.vector.tensor_mul(pt, pt, masks)
                ot = ots[g]
                for h in range(2):
                    j = 2 * i + h
                    nc.tensor.matmul(ot, lhsT=v_sb[:, j, :],
                                     rhs=pt[:, h * QG:(h + 1) * QG],
                                     start=(i == 0 and h == 0),
                                     stop=(i == g and h == 1))
                if i == g:
                    # free the PSUM accumulator promptly, then DMA from SBUF
                    nc.vector.tensor_copy(out_sb[:, g, :], ot)
                    nc.sync.dma_start(out=out[:, g, :], in_=out_sb[:, g, :])
                    del ots[g]

            # input DMAs issued up-front, ordered by when the data is first
            # needed; the big SBUF tiles are persistent so transfers stream
            # in while compute chases the data.
            H = QG // 2
            dma_q(0)
            dma_k(0, H)
            nc.sync.dma_start(out=thr_sb, in_=thr)
            dma_v(0, H)
            dma_q(1)
            dma_k(H, H)
            dma_q(2)
            dma_v(H, H)
            dma_k(QG, QG)
            dma_v(QG, QG)
            dma_q(3)
            dma_q(4)
            dma_k(2 * QG, QG)
            dma_v(2 * QG, QG)
            dma_q(5)
            dma_q(6)
            dma_k(3 * QG, QG)
            dma_v(3 * QG, QG)
            dma_q(7)

            # deferred K/V projection work, keyed by the pair-iteration just
            # before the data's first consumer (in-order engine queues: work
            # emitted too early blocks ready work queued behind it).
            def T(m):
                return m * (m + 1) // 2

            work_at = {}

            def at(n, fn, *a):
                work_at.setdefault(max(0, n), []).append((fn, a))

            for kp in (1, 2, 3):
                at(T(2 * kp) + 2 * kp - 2, proj_k, kp * QG, QG)
            for blk in range(4, 4 * 4):
                at(T(blk // 2) + blk // 2 - 2, proj_v, blk)

            def prologue(g):
                """Q projection + accumulator for slot g."""
                proj_q(g)
                ots[g] = pacc.tile([DH + 1, QG], f32, tag="acc", name="ot")

            # head: minimal chain to the first exp
            proj_q(0)
            proj_k(0, QG // 2, on_act=True)
            ots[0] = pacc.tile([DH + 1, QG], f32, tag="acc", name="ot")
            stage_A(0)
            proj_k(QG // 2, QG // 2)
            proj_v(0)
            proj_v(1)
            at(0, proj_v, 2)
            at(0, proj_v, 3)
            prologue(1)
            for n in range(npairs):
                g, i = pairs[n]
                if n + 1 < npairs:
                    stage_A(n + 1)
                if i == 0 and 1 <= g < NSLOTS - 1:
                    prologue(g + 1)
                for fn, a in work_at.get(n, ()):
                    fn(*a)
                stage_EP(n)

    nc.compile()
    return nc


def _get_program():
    global _PROGRAM
    if _PROGRAM is None:
        _PROGRAM = _build_program()
    return _PROGRAM


def _thr_table(p):
    t = np.empty((128, 2), np.float32)
    x = np.arange(128, dtype=np.float32)
    t[:, 0] = 128 * p + x
    t[:, 1] = 128 * (2 + p) + x
    return t


def kernel(q, k, v, Wq, Wk, Wv, mask):
    assert q.shape == (B, S, DM) and k.shape == (B, S, DM)
    assert v.shape == (B, S, DM) and Wq.shape == (DM, DH)

    nc = _get_program()
    from concourse.bass_utils import run_bass_kernel_spmd

    q = np.asarray(q, np.float32)
    k = np.asarray(k, np.float32)
    v = np.asarray(v, np.float32)
    w3 = np.concatenate([np.asarray(Wq, np.float32),
                         np.asarray(Wk, np.float32),
                         np.asarray(Wv, np.float32)], axis=1).astype(BF16)
    # pre-arrange to the SBUF layout [128 partitions, (dm_chunk, 3*dh)]
    w3 = np.ascontiguousarray(
        w3.reshape(4, 128, 3 * DH).transpose(1, 0, 2).reshape(128, 12 * DH))

    qT_b = [np.ascontiguousarray(q[b].T).astype(BF16) for b in range(B)]

    in_maps = []
    for c in range(NCORES):
        b, p = c % 4, c // 4
        k_sel = k[b].reshape(S // 128, 128, DM)[p::2].reshape(S // 2, DM)
        v_sel = v[b].reshape(S // 128, 128, DM)[p::2].reshape(S // 2, DM)
        in_maps.append({
            "qT": qT_b[b],
            "kT": np.ascontiguousarray(k_sel.T).astype(BF16),
            "vT": np.ascontiguousarray(v_sel.T).astype(BF16),
            "w3": w3,
            "thr": _thr_table(p),
        })

    res = run_bass_kernel_spmd(nc, in_maps, core_ids=list(range(NCORES)))

    O = np.empty((B, S, DH), np.float32)
    for b in range(B):
        tot = res.results[b]["out"] + res.results[b + 4]["out"]
        denom = tot[DH]                       # [8, 512]
        o = tot[0:DH].transpose(1, 2, 0)      # [8, 512, 64]
        O[b] = (o / denom[:, :, None]).reshape(S, DH)
    return O


# revision 31
# speedup vs baseline: 1.0376x; 1.0376x over previous
"""Single-head causal attention (B=4, S=4096, d_model=512, d_head=64) on 8 trn2 cores.

Sharding: core c handles batch c%4 and the key-chunk parity p=c//4 — it sees
ALL 4096 queries of its batch but only the 16 key-chunks (of 128 keys) with
global index % 2 == p.  Each core produces the unnormalized partial
attention O'^T = V'^T P^T (with a ones-column giving per-query partial
softmax denominators in row 64); the host adds the two partials per batch
and normalizes.  This is exactly softmax split over keys, and it makes the
program perfectly uniform: every core runs slots g=0..7 (512 queries each)
with exactly 2g+2 local key chunks — no padding, no garbage work.

Causality: local chunk j of slot g covers global keys 128*(2j+p)+x; chunks
j <= 2g-1 are fully attended, and only the last two chunks (j=2g, 2g+1) are
masked.  The mask threshold 128*(2t+p)+x is slot-independent, so a single
[128, 1024] mask (computed on-chip from a 2-column threshold table) covers
every slot's final pair.

Device algorithm (bf16 data, fp32 accumulation):
  QT = Wq^T qT  [64, 4096]  (PE, psum->sbuf copies on DVE)
  KT = Wk^T kT  [64, 2048]
  V  = (vT chunks)^T Wv row-form [128, 16, 64] + ones column -> V' [128,16,65]
  per slot g, chunk pair i (software-pipelined one pair ahead):
     S^T[:, 0:512]   = K_{2i} Q_g^T   (PSUM [128, 1024] two banks)
     S^T[:, 512:1024]= K_{2i+1} Q_g^T
     P^T = exp(S^T/8)  (one ACT instruction over [128, 1024] -> bf16)
     if last pair: P^T *= mask (DVE, 4x mode)
     O'[65, 512] += V'_{2i}^T P^T_lo + V'_{2i+1}^T P^T_hi  (PE)
  O' slots DMA'd straight from PSUM to DRAM; host combines + normalizes.
"""

import os
import sys

import numpy as np

for _p in ("/opt/trn_rl_repo",):
    if _p not in sys.path and os.path.isdir(_p):
        sys.path.insert(0, _p)

import ml_dtypes

B, S, DM, DH = 4, 4096, 512, 64
NCORES = 8
NSLOTS = 8                    # query slots of 512 rows (full batch per core)
QG = 512
NKC_LOC = 16                  # local key chunks of 128 keys per core

BF16 = ml_dtypes.bfloat16

_PROGRAM = None


def _build_program():
    import concourse.bacc as bacc
    import concourse.mybir as mybir
    import concourse.tile as tile

    f32 = mybir.dt.float32
    bf16 = mybir.dt.bfloat16
    fp8 = mybir.dt.float8e4
    Exp = mybir.ActivationFunctionType.Exp
    DoubleRow = mybir.MatmulPerfMode.DoubleRow

    nc = bacc.Bacc("TRN2", target_bir_lowering=False, debug=False,
                   num_devices=NCORES)

    qT = nc.dram_tensor("qT", [DM, S], bf16, kind="ExternalInput").ap()
    kT = nc.dram_tensor("kT", [DM, S // 2], bf16, kind="ExternalInput").ap()
    vT = nc.dram_tensor("vT", [DM, S // 2], bf16, kind="ExternalInput").ap()
    # wq | wk | wv concatenated, pre-arranged on host to the SBUF layout
    # [128, (dm_chunk, 3*dh)] so the DMA has >=512B contiguous runs
    w3 = nc.dram_tensor("w3", [128, 4 * 3 * DH], bf16, kind="ExternalInput").ap()
    # thr[x, t] = 128*(2t + parity) + x : q-column threshold for the two
    # masked chunks of every slot's final pair
    thr = nc.dram_tensor("thr", [128, 2], f32, kind="ExternalInput").ap()
    # out[d, g, q] : rows 0..63 = unnormalized O'^T, row 64 = partial denom
    out = nc.dram_tensor("out", [DH + 1, NSLOTS, QG], f32,
                         kind="ExternalOutput").ap()

    qT_r = qT.rearrange("(c p) n -> p c n", p=128)
    kT_r = kT.rearrange("(c p) n -> p c n", p=128)
    vT_r = vT.rearrange("(c p) n -> p c n", p=128)

    with tile.TileContext(nc) as tc:
        with (
            tc.tile_pool(name="consts", bufs=1) as consts,
            tc.tile_pool(name="big", bufs=1) as big,
            tc.tile_pool(name="work", bufs=3) as work,
            tc.tile_pool(name="pst", bufs=2, space="PSUM") as pst,
            tc.tile_pool(name="pacc", bufs=2, space="PSUM") as pacc,
            tc.tile_pool(name="psm", bufs=2, space="PSUM") as psm,
        ):
            # ---- constants ----
            w_sb = consts.tile([128, 4, 3 * DH], bf16)
            nc.sync.dma_start(out=w_sb,
                              in_=w3.rearrange("p (c d) -> p c d", c=4))

            def wq_sb(c):
                return w_sb[:, c, 0:DH]

            def wk_sb(c):
                return w_sb[:, c, DH:2 * DH]

            def wv_sb(c):
                return w_sb[:, c, 2 * DH:3 * DH]

            thr_sb = consts.tile([128, 2], f32)
            yiota = consts.tile([128, QG], f32)
            masks = consts.tile([128, 2 * QG], bf16)

            def gen_masks():
                # on gpsimd: keeps the DVE queue free for projection copies.
                # Emitted after the fp8 zero memsets (Pool queue is in-order
                # and the is_ge waits on the thr DMA).
                nc.gpsimd.iota(yiota, pattern=[[1, QG]], base=0,
                               channel_multiplier=0,
                               allow_small_or_imprecise_dtypes=True)
                for t in range(2):
                    nc.gpsimd.tensor_scalar(
                        out=masks[:, t * QG:(t + 1) * QG], in0=yiota,
                        scalar1=thr_sb[:, t:t + 1], scalar2=None,
                        op0=mybir.AluOpType.is_ge)

            # ---- staged inputs / projected tensors ----
            qT_sb = big.tile([128, 4, S], bf16)
            kT_sb = big.tile([128, 4, S // 2], bf16)
            vT_sb = big.tile([128, 4, S // 2], bf16)
            # Q/K projected then quantized to fp8 for DoubleRow score matmuls;
            # layout [64, 2, n]: half 0 = real 64-dim values, half 1 = zeros
            # (DoubleRow contracts over both halves; the zero half is free and
            # lets the fp8 conversion be a single 64-partition copy)
            q8_sb = big.tile([64, 2, S], fp8)
            k8_sb = big.tile([64, 2, S // 2], fp8)
            # zero halves in pieces, first-needed first, so early score
            # matmuls aren't gated on one huge memset
            nc.gpsimd.memset(k8_sb[:, 1, 0:QG], 0.0)
            nc.gpsimd.memset(q8_sb[:, 1, 0:2 * QG], 0.0)
            gen_masks()
            for kp in range(1, 4):
                nc.gpsimd.memset(k8_sb[:, 1, kp * QG:(kp + 1) * QG], 0.0)
            for gp in range(1, 4):
                nc.gpsimd.memset(
                    q8_sb[:, 1, 2 * gp * QG:2 * (gp + 1) * QG], 0.0)
            v_sb = big.tile([128, NKC_LOC, DH + 1], bf16)
            nc.vector.memset(v_sb[:, :, DH:DH + 1], 1.0)
            out_sb = big.tile([DH + 1, NSLOTS, QG], f32)

            def dma_q(g):
                nc.sync.dma_start(out=qT_sb[:, :, g * QG:(g + 1) * QG],
                                  in_=qT_r[:, :, g * QG:(g + 1) * QG])

            def dma_k(c0, n):
                sl = slice(c0, c0 + n)
                nc.sync.dma_start(out=kT_sb[:, :, sl], in_=kT_r[:, :, sl])

            def dma_v(c0, n):
                sl = slice(c0, c0 + n)
                nc.sync.dma_start(out=vT_sb[:, :, sl], in_=vT_r[:, :, sl])

            def proj_q(g):
                sl = slice(g * QG, (g + 1) * QG)
                pp = psm.tile([64, QG], f32, tag="sm", name="pq")
                for c in range(4):
                    nc.tensor.matmul(pp, lhsT=wq_sb(c),
                                     rhs=qT_sb[:, c, sl],
                                     start=(c == 0), stop=(c == 3))
                nc.vector.tensor_copy(q8_sb[:, 0, sl], pp)

            def proj_k(col0, ncols, on_act=False):
                sl = slice(col0, col0 + ncols)
                pp = psm.tile([64, QG], f32, tag="sm", name="pk")
                for c in range(4):
                    nc.tensor.matmul(pp[:, 0:ncols], lhsT=wk_sb(c),
                                     rhs=kT_sb[:, c, sl],
                                     start=(c == 0), stop=(c == 3))
                if on_act:  # head only: ACT is idle before the first exp
                    nc.scalar.copy(k8_sb[:, 0, sl], pp[:, 0:ncols])
                else:
                    nc.vector.tensor_copy(k8_sb[:, 0, sl], pp[:, 0:ncols])

            def proj_v(kb):
                pv = psm.tile([128, DH], f32, tag="sm", name="pv")
                for c in range(4):
                    nc.tensor.matmul(pv,
                                     lhsT=vT_sb[:, c, kb * 128:(kb + 1) * 128],
                                     rhs=wv_sb(c),
                                     start=(c == 0), stop=(c == 3))
                nc.vector.tensor_copy(v_sb[:, kb, 0:DH], pv)

            # ---- software-pipelined main sequence ----
            # pairs[n] = (slot g, pair index i); stage A (scores) is emitted
            # one pair ahead of stage EP (exp + PV accumulate) so the PE has
            # score work queued while ACT runs the previous pair's exp.
            pairs = [(g, i) for g in range(NSLOTS) for i in range(g + 1)]
            npairs = len(pairs)

            sts = {}
            ots = {}

            def stage_A(n):
                g, i = pairs[n]
                st = pst.tile([128, 2 * QG], f32, tag="st", name="st")
                sts[n] = st
                qcols = q8_sb[:, :, g * QG:(g + 1) * QG]
                for h in range(2):
                    j = 2 * i + h
                    nc.tensor.matmul(st[:, h * QG:(h + 1) * QG],
                                     lhsT=k8_sb[:, :, j * 128:(j + 1) * 128],
                                     rhs=qcols, start=True, stop=True,
                                     perf_mode=DoubleRow)

            def stage_EP(n):
                g, i = pairs[n]
                st = sts.pop(n)
                pt = work.tile([128, 2 * QG], bf16, tag="pt", name="pt")
                nc.scalar.activation(pt, st, Exp, scale=0.125)
                if i == g:
                    nc.vector.tensor_mul(pt, pt, masks)
                ot = ots[g]
                for h in range(2):
                    j = 2 * i + h
                    nc.tensor.matmul(ot, lhsT=v_sb[:, j, :],
                                     rhs=pt[:, h * QG:(h + 1) * QG],
                                     start=(i == 0 and h == 0),
                                     stop=(i == g and h == 1))
                if i == g:
                    # free the PSUM accumulator promptly, then DMA from SBUF
                    nc.vector.tensor_copy(out_sb[:, g, :], ot)
                    nc.sync.dma_start(out=out[:, g, :], in_=out_sb[:, g, :])
                    del ots[g]

            # input DMAs issued up-front, ordered by when the data is first
            # needed; the big SBUF tiles are persistent so transfers stream
            # in while compute chases the data.
            H = QG // 2
            dma_q(0)
            dma_k(0, H)
            dma_k(H, H)
            dma_v(0, H)
            nc.sync.dma_start(out=thr_sb, in_=thr)
            dma_q(1)
            dma_q(2)
            dma_v(H, H)
            dma_k(QG, QG)
            dma_v(QG, QG)
            dma_q(3)
            dma_q(4)
            dma_k(2 * QG, QG)
            dma_v(2 * QG, QG)
            dma_q(5)
            dma_q(6)
            dma_k(3 * QG, QG)
            dma_v(3 * QG, QG)
            dma_q(7)

            # deferred K/V projection work, keyed by the pair-iteration just
            # before the data's first consumer (in-order engine queues: work
            # emitted too early blocks ready work queued behind it).
            def T(m):
                return m * (m + 1) // 2

            work_at = {}

            def at(n, fn, *a):
                work_at.setdefault(max(0, n), []).append((fn, a))

            def prologue(g):
                """Q projection + accumulator for slot g."""
                proj_q(g)
                ots[g] = pacc.tile([DH + 1, QG], f32, tag="acc", name="ot")

            # Deadlines (pair index whose EP precedes the emission).  Chosen
            # so same-engine priority order matches data-arrival order: a
            # Q/K projection emitted before its DMA lands gets its (always
            # ready) Ldweights popped by the idle PE and the matmul behind
            # it head-blocks the engine queue.
            for g2, nn in {2: 1, 3: 4, 4: 6, 5: 10, 6: 12, 7: 16}.items():
                at(nn, prologue, g2)
            at(4, proj_k, QG, QG)
            at(12, proj_k, 2 * QG, QG)
            at(25, proj_k, 3 * QG, QG)
            for blk in range(4, 4 * 4):
                at(T(blk // 2) + blk // 2 - 2, proj_v, blk)

            # head: minimal chain to the first exp
            proj_q(0)
            proj_k(0, QG // 2, on_act=True)
            ots[0] = pacc.tile([DH + 1, QG], f32, tag="acc", name="ot")
            stage_A(0)
            proj_k(QG // 2, QG // 2)
            proj_v(0)
            proj_v(1)
            at(0, proj_v, 2)
            at(0, proj_v, 3)
            prologue(1)
            for n in range(npairs):
                g, i = pairs[n]
                if n + 1 < npairs:
                    stage_A(n + 1)
                stage_EP(n)
                for fn, a in work_at.get(n, ()):
                    fn(*a)

    nc.compile()
    return nc


def _get_program():
    global _PROGRAM
    if _PROGRAM is None:
        _PROGRAM = _build_program()
    return _PROGRAM


def _thr_table(p):
    t = np.empty((128, 2), np.float32)
    x = np.arange(128, dtype=np.float32)
    t[:, 0] = 128 * p + x
    t[:, 1] = 128 * (2 + p) + x
    return t


def kernel(q, k, v, Wq, Wk, Wv, mask):
    assert q.shape == (B, S, DM) and k.shape == (B, S, DM)
    assert v.shape == (B, S, DM) and Wq.shape == (DM, DH)

    nc = _get_program()
    from concourse.bass_utils import run_bass_kernel_spmd

    q = np.asarray(q, np.float32)
    k = np.asarray(k, np.float32)
    v = np.asarray(v, np.float32)
    w3 = np.concatenate([np.asarray(Wq, np.float32),
                         np.asarray(Wk, np.float32),
                         np.asarray(Wv, np.float32)], axis=1).astype(BF16)
    # pre-arrange to the SBUF layout [128 partitions, (dm_chunk, 3*dh)]
    w3 = np.ascontiguousarray(
        w3.reshape(4, 128, 3 * DH).transpose(1, 0, 2).reshape(128, 12 * DH))

    qT_b = [np.ascontiguousarray(q[b].T).astype(BF16) for b in range(B)]

    in_maps = []
    for c in range(NCORES):
        b, p = c % 4, c // 4
        k_sel = k[b].reshape(S // 128, 128, DM)[p::2].reshape(S // 2, DM)
        v_sel = v[b].reshape(S // 128, 128, DM)[p::2].reshape(S // 2, DM)
        in_maps.append({
            "qT": qT_b[b],
            "kT": np.ascontiguousarray(k_sel.T).astype(BF16),
            "vT": np.ascontiguousarray(v_sel.T).astype(BF16),
            "w3": w3,
            "thr": _thr_table(p),
        })

    res = run_bass_kernel_spmd(nc, in_maps, core_ids=list(range(NCORES)))

    O = np.empty((B, S, DH), np.float32)
    for b in range(B):
        tot = res.results[b]["out"] + res.results[b + 4]["out"]
        denom = tot[DH]                       # [8, 512]
        o = tot[0:DH].transpose(1, 2, 0)      # [8, 512, 64]
        O[b] = (o / denom[:, :, None]).reshape(S, DH)
    return O


# revision 35
# speedup vs baseline: 1.0558x; 1.0175x over previous
"""Single-head causal attention (B=4, S=4096, d_model=512, d_head=64) on 8 trn2 cores.

Sharding: core c handles batch c%4 and the key-chunk parity p=c//4 — it sees
ALL 4096 queries of its batch but only the 16 key-chunks (of 128 keys) with
global index % 2 == p.  Each core produces the unnormalized partial
attention O'^T = V'^T P^T (with a ones-column giving per-query partial
softmax denominators in row 64); the host adds the two partials per batch
and normalizes.  This is exactly softmax split over keys, and it makes the
program perfectly uniform: every core runs slots g=0..7 (512 queries each)
with exactly 2g+2 local key chunks — no padding, no garbage work.

Causality: local chunk j of slot g covers global keys 128*(2j+p)+x; chunks
j <= 2g-1 are fully attended, and only the last two chunks (j=2g, 2g+1) are
masked.  The mask threshold 128*(2t+p)+x is slot-independent, so a single
[128, 1024] mask (computed on-chip from a 2-column threshold table) covers
every slot's final pair.

Device algorithm (bf16 data, fp32 accumulation):
  QT = Wq^T qT  [64, 4096]  (PE, psum->sbuf copies on DVE)
  KT = Wk^T kT  [64, 2048]
  V  = (vT chunks)^T Wv row-form [128, 16, 64] + ones column -> V' [128,16,65]
  per slot g, chunk pair i (software-pipelined one pair ahead):
     S^T[:, 0:512]   = K_{2i} Q_g^T   (PSUM [128, 1024] two banks)
     S^T[:, 512:1024]= K_{2i+1} Q_g^T
     P^T = exp(S^T/8)  (one ACT instruction over [128, 1024] -> bf16)
     if last pair: P^T *= mask (DVE, 4x mode)
     O'[65, 512] += V'_{2i}^T P^T_lo + V'_{2i+1}^T P^T_hi  (PE)
  O' slots DMA'd straight from PSUM to DRAM; host combines + normalizes.
"""

import os
import sys

import numpy as np

for _p in ("/opt/trn_rl_repo",):
    if _p not in sys.path and os.path.isdir(_p):
        sys.path.insert(0, _p)

import ml_dtypes

B, S, DM, DH = 4, 4096, 512, 64
NCORES = 8
NSLOTS = 8                    # query slots of 512 rows (full batch per core)
QG = 512
NKC_LOC = 16                  # local key chunks of 128 keys per core

BF16 = ml_dtypes.bfloat16

_PROGRAM = None


def _build_program():
    import concourse.bacc as bacc
    import concourse.mybir as mybir
    import concourse.tile as tile

    f32 = mybir.dt.float32
    bf16 = mybir.dt.bfloat16
    fp8 = mybir.dt.float8e4
    Exp = mybir.ActivationFunctionType.Exp
    DoubleRow = mybir.MatmulPerfMode.DoubleRow

    nc = bacc.Bacc("TRN2", target_bir_lowering=False, debug=False,
                   num_devices=NCORES)

    qT = nc.dram_tensor("qT", [DM, S], bf16, kind="ExternalInput").ap()
    kT = nc.dram_tensor("kT", [DM, S // 2], bf16, kind="ExternalInput").ap()
    vT = nc.dram_tensor("vT", [DM, S // 2], bf16, kind="ExternalInput").ap()
    # wq | wk | wv concatenated, pre-arranged on host to the SBUF layout
    # [128, (dm_chunk, 3*dh)] so the DMA has >=512B contiguous runs
    w3 = nc.dram_tensor("w3", [128, 4 * 3 * DH], bf16, kind="ExternalInput").ap()
    # thr[x, t] = 128*(2t + parity) + x : q-column threshold for the two
    # masked chunks of every slot's final pair
    thr = nc.dram_tensor("thr", [128, 2], f32, kind="ExternalInput").ap()
    # out[d, g, q] : rows 0..63 = unnormalized O'^T, row 64 = partial denom
    out = nc.dram_tensor("out", [DH + 1, NSLOTS, QG], f32,
                         kind="ExternalOutput").ap()

    qT_r = qT.rearrange("(c p) n -> p c n", p=128)
    kT_r = kT.rearrange("(c p) n -> p c n", p=128)
    vT_r = vT.rearrange("(c p) n -> p c n", p=128)

    with tile.TileContext(nc) as tc:
        with (
            tc.tile_pool(name="consts", bufs=1) as consts,
            tc.tile_pool(name="big", bufs=1) as big,
            tc.tile_pool(name="work", bufs=3) as work,
            tc.tile_pool(name="pst", bufs=2, space="PSUM") as pst,
            tc.tile_pool(name="pacc", bufs=2, space="PSUM") as pacc,
            tc.tile_pool(name="psm", bufs=2, space="PSUM") as psm,
        ):
            # ---- constants ----
            w_sb = consts.tile([128, 4, 3 * DH], bf16)
            nc.sync.dma_start(out=w_sb,
                              in_=w3.rearrange("p (c d) -> p c d", c=4))

            def wq_sb(c):
                return w_sb[:, c, 0:DH]

            def wk_sb(c):
                return w_sb[:, c, DH:2 * DH]

            def wv_sb(c):
                return w_sb[:, c, 2 * DH:3 * DH]

            thr_sb = consts.tile([128, 2], f32)
            yiota = consts.tile([128, QG], f32)
            masks = consts.tile([128, 2 * QG], bf16)

            def gen_masks():
                # on gpsimd: keeps the DVE queue free for projection copies.
                # Emitted after the fp8 zero memsets (Pool queue is in-order
                # and the is_ge waits on the thr DMA).
                nc.gpsimd.iota(yiota, pattern=[[1, QG]], base=0,
                               channel_multiplier=0,
                               allow_small_or_imprecise_dtypes=True)
                for t in range(2):
                    nc.gpsimd.tensor_scalar(
                        out=masks[:, t * QG:(t + 1) * QG], in0=yiota,
                        scalar1=thr_sb[:, t:t + 1], scalar2=None,
                        op0=mybir.AluOpType.is_ge)

            # ---- staged inputs / projected tensors ----
            qT_sb = big.tile([128, 4, S], bf16)
            kT_sb = big.tile([128, 4, S // 2], bf16)
            vT_sb = big.tile([128, 4, S // 2], bf16)
            # Q/K projected then quantized to fp8 for DoubleRow score matmuls;
            # layout [64, 2, n]: half 0 = real 64-dim values, half 1 = zeros
            # (DoubleRow contracts over both halves; the zero half is free and
            # lets the fp8 conversion be a single 64-partition copy)
            q8_sb = big.tile([64, 2, S], fp8)
            k8_sb = big.tile([64, 2, S // 2], fp8)
            # zero halves in pieces, first-needed first, so early score
            # matmuls aren't gated on one huge memset
            nc.gpsimd.memset(k8_sb[:, 1, 0:QG], 0.0)
            nc.gpsimd.memset(q8_sb[:, 1, 0:2 * QG], 0.0)
            gen_masks()
            for kp in range(1, 4):
                nc.gpsimd.memset(k8_sb[:, 1, kp * QG:(kp + 1) * QG], 0.0)
            for gp in range(1, 4):
                nc.gpsimd.memset(
                    q8_sb[:, 1, 2 * gp * QG:2 * (gp + 1) * QG], 0.0)
            v_sb = big.tile([128, NKC_LOC, DH + 1], bf16)
            nc.vector.memset(v_sb[:, :, DH:DH + 1], 1.0)
            out_sb = big.tile([DH + 1, NSLOTS, QG], f32)

            def dma_q(g):
                nc.sync.dma_start(out=qT_sb[:, :, g * QG:(g + 1) * QG],
                                  in_=qT_r[:, :, g * QG:(g + 1) * QG])

            def dma_k(c0, n):
                sl = slice(c0, c0 + n)
                nc.sync.dma_start(out=kT_sb[:, :, sl], in_=kT_r[:, :, sl])

            def dma_v(c0, n):
                sl = slice(c0, c0 + n)
                nc.sync.dma_start(out=vT_sb[:, :, sl], in_=vT_r[:, :, sl])

            def proj_q(g):
                sl = slice(g * QG, (g + 1) * QG)
                pp = psm.tile([64, QG], f32, tag="sm", name="pq")
                for c in range(4):
                    nc.tensor.matmul(pp, lhsT=wq_sb(c),
                                     rhs=qT_sb[:, c, sl],
                                     start=(c == 0), stop=(c == 3))
                nc.vector.tensor_copy(q8_sb[:, 0, sl], pp)

            def proj_k(col0, ncols, on_act=False):
                sl = slice(col0, col0 + ncols)
                pp = psm.tile([64, QG], f32, tag="sm", name="pk")
                for c in range(4):
                    nc.tensor.matmul(pp[:, 0:ncols], lhsT=wk_sb(c),
                                     rhs=kT_sb[:, c, sl],
                                     start=(c == 0), stop=(c == 3))
                if on_act:  # head only: ACT is idle before the first exp
                    nc.scalar.copy(k8_sb[:, 0, sl], pp[:, 0:ncols])
                else:
                    nc.vector.tensor_copy(k8_sb[:, 0, sl], pp[:, 0:ncols])

            def proj_v(kb):
                pv = psm.tile([128, DH], f32, tag="sm", name="pv")
                for c in range(4):
                    nc.tensor.matmul(pv,
                                     lhsT=vT_sb[:, c, kb * 128:(kb + 1) * 128],
                                     rhs=wv_sb(c),
                                     start=(c == 0), stop=(c == 3))
                nc.vector.tensor_copy(v_sb[:, kb, 0:DH], pv)

            # ---- software-pipelined main sequence ----
            # pairs[n] = (slot g, chunk-pair j, first, last, masked); stage A
            # (scores) is emitted one pair ahead of stage EP (exp + PV
            # accumulate) so the PE has score work queued while ACT runs the
            # previous pair's exp.  The last slot processes its masked
            # (diagonal) pair FIRST so the final tail chain skips the mask.
            pairs = []
            for g in range(NSLOTS):
                order = [g] + list(range(g)) if g == NSLOTS - 1 \
                    else list(range(g + 1))
                for idx, j in enumerate(order):
                    pairs.append((g, j, idx == 0, idx == len(order) - 1,
                                  j == g))
            npairs = len(pairs)

            sts = {}
            ots = {}

            def stage_A(n):
                g, jp, _, _, _ = pairs[n]
                st = pst.tile([128, 2 * QG], f32, tag="st", name="st")
                sts[n] = st
                qcols = q8_sb[:, :, g * QG:(g + 1) * QG]
                for h in range(2):
                    j = 2 * jp + h
                    nc.tensor.matmul(st[:, h * QG:(h + 1) * QG],
                                     lhsT=k8_sb[:, :, j * 128:(j + 1) * 128],
                                     rhs=qcols, start=True, stop=True,
                                     perf_mode=DoubleRow)

            def stage_EP(n):
                g, jp, first, last, masked = pairs[n]
                st = sts.pop(n)
                pt = work.tile([128, 2 * QG], bf16, tag="pt", name="pt")
                nc.scalar.activation(pt, st, Exp, scale=0.125)
                if masked:
                    nc.vector.tensor_mul(pt, pt, masks)
                ot = ots[g]
                for h in range(2):
                    j = 2 * jp + h
                    nc.tensor.matmul(ot, lhsT=v_sb[:, j, :],
                                     rhs=pt[:, h * QG:(h + 1) * QG],
                                     start=(first and h == 0),
                                     stop=(last and h == 1))
                if last:
                    # free the PSUM accumulator promptly, then DMA from SBUF
                    nc.vector.tensor_copy(out_sb[:, g, :], ot)
                    nc.sync.dma_start(out=out[:, g, :], in_=out_sb[:, g, :])
                    del ots[g]

            # input DMAs issued up-front, ordered by when the data is first
            # needed; the big SBUF tiles are persistent so transfers stream
            # in while compute chases the data.
            H = QG // 2
            dma_q(0)
            dma_k(0, H)
            dma_q(1)
            dma_k(H, H)
            dma_v(0, H)
            nc.sync.dma_start(out=thr_sb, in_=thr)
            dma_q(2)
            dma_v(H, H)
            dma_k(QG, QG)
            dma_v(QG, QG)
            dma_q(3)
            dma_q(4)
            dma_k(2 * QG, QG)
            dma_v(2 * QG, QG)
            dma_q(5)
            dma_q(6)
            dma_k(3 * QG, QG)
            dma_v(3 * QG, QG)
            dma_q(7)

            # deferred K/V projection work, keyed by the pair-iteration just
            # before the data's first consumer (in-order engine queues: work
            # emitted too early blocks ready work queued behind it).
            def T(m):
                return m * (m + 1) // 2

            work_at = {}

            def at(n, fn, *a):
                work_at.setdefault(max(0, n), []).append((fn, a))

            def prologue(g):
                """Q projection + accumulator for slot g."""
                proj_q(g)
                ots[g] = pacc.tile([DH + 1, QG], f32, tag="acc", name="ot")

            # Deadlines (pair index whose EP precedes the emission).  Chosen
            # so same-engine priority order matches data-arrival order: a
            # Q/K projection emitted before its DMA lands gets its (always
            # ready) Ldweights popped by the idle PE and the matmul behind
            # it head-blocks the engine queue.
            for g2, nn in {2: 1, 3: 4, 4: 5, 5: 8, 6: 9, 7: 13}.items():
                at(nn, prologue, g2)
            at(3, proj_k, QG, QG)
            at(7, proj_k, 2 * QG, QG)
            at(11, proj_k, 3 * QG, QG)
            for blk in range(4, 4 * 4):
                at(T(blk // 2) + blk // 2 - 2, proj_v, blk)

            # head: minimal chain to the first exp
            proj_q(0)
            proj_k(0, QG // 2, on_act=True)
            ots[0] = pacc.tile([DH + 1, QG], f32, tag="acc", name="ot")
            stage_A(0)
            proj_k(QG // 2, QG // 2)
            proj_v(0)
            proj_v(1)
            at(0, proj_v, 2)
            at(0, proj_v, 3)
            prologue(1)
            for n in range(npairs):
                if n + 1 < npairs:
                    stage_A(n + 1)
                stage_EP(n)
                for fn, a in work_at.get(n, ()):
                    fn(*a)

    nc.compile()
    return nc


def _get_program():
    global _PROGRAM
    if _PROGRAM is None:
        _PROGRAM = _build_program()
    return _PROGRAM


def _thr_table(p):
    t = np.empty((128, 2), np.float32)
    x = np.arange(128, dtype=np.float32)
    t[:, 0] = 128 * p + x
    t[:, 1] = 128 * (2 + p) + x
    return t


def kernel(q, k, v, Wq, Wk, Wv, mask):
    assert q.shape == (B, S, DM) and k.shape == (B, S, DM)
    assert v.shape == (B, S, DM) and Wq.shape == (DM, DH)

    nc = _get_program()
    from concourse.bass_utils import run_bass_kernel_spmd

    q = np.asarray(q, np.float32)
    k = np.asarray(k, np.float32)
    v = np.asarray(v, np.float32)
    w3 = np.concatenate([np.asarray(Wq, np.float32),
                         np.asarray(Wk, np.float32),
                         np.asarray(Wv, np.float32)], axis=1).astype(BF16)
    # pre-arrange to the SBUF layout [128 partitions, (dm_chunk, 3*dh)]
    w3 = np.ascontiguousarray(
        w3.reshape(4, 128, 3 * DH).transpose(1, 0, 2).reshape(128, 12 * DH))

    qT_b = [np.ascontiguousarray(q[b].T).astype(BF16) for b in range(B)]

    in_maps = []
    for c in range(NCORES):
        b, p = c % 4, c // 4
        k_sel = k[b].reshape(S // 128, 128, DM)[p::2].reshape(S // 2, DM)
        v_sel = v[b].reshape(S // 128, 128, DM)[p::2].reshape(S // 2, DM)
        in_maps.append({
            "qT": qT_b[b],
            "kT": np.ascontiguousarray(k_sel.T).astype(BF16),
            "vT": np.ascontiguousarray(v_sel.T).astype(BF16),
            "w3": w3,
            "thr": _thr_table(p),
        })

    res = run_bass_kernel_spmd(nc, in_maps, core_ids=list(range(NCORES)))

    O = np.empty((B, S, DH), np.float32)
    for b in range(B):
        tot = res.results[b]["out"] + res.results[b + 4]["out"]
        denom = tot[DH]                       # [8, 512]
        o = tot[0:DH].transpose(1, 2, 0)      # [8, 512, 64]
        O[b] = (o / denom[:, :, None]).reshape(S, DH)
    return O


# revision 37
# speedup vs baseline: 1.0628x; 1.0066x over previous
"""Single-head causal attention (B=4, S=4096, d_model=512, d_head=64) on 8 trn2 cores.

Sharding: core c handles batch c%4 and the key-chunk parity p=c//4 — it sees
ALL 4096 queries of its batch but only the 16 key-chunks (of 128 keys) with
global index % 2 == p.  Each core produces the unnormalized partial
attention O'^T = V'^T P^T (with a ones-column giving per-query partial
softmax denominators in row 64); the host adds the two partials per batch
and normalizes.  This is exactly softmax split over keys, and it makes the
program perfectly uniform: every core runs slots g=0..7 (512 queries each)
with exactly 2g+2 local key chunks — no padding, no garbage work.

Causality: local chunk j of slot g covers global keys 128*(2j+p)+x; chunks
j <= 2g-1 are fully attended, and only the last two chunks (j=2g, 2g+1) are
masked.  The mask threshold 128*(2t+p)+x is slot-independent, so a single
[128, 1024] mask (computed on-chip from a 2-column threshold table) covers
every slot's final pair.

Device algorithm (bf16 data, fp32 accumulation):
  QT = Wq^T qT  [64, 4096]  (PE, psum->sbuf copies on DVE)
  KT = Wk^T kT  [64, 2048]
  V  = (vT chunks)^T Wv row-form [128, 16, 64] + ones column -> V' [128,16,65]
  per slot g, chunk pair i (software-pipelined one pair ahead):
     S^T[:, 0:512]   = K_{2i} Q_g^T   (PSUM [128, 1024] two banks)
     S^T[:, 512:1024]= K_{2i+1} Q_g^T
     P^T = exp(S^T/8)  (one ACT instruction over [128, 1024] -> bf16)
     if last pair: P^T *= mask (DVE, 4x mode)
     O'[65, 512] += V'_{2i}^T P^T_lo + V'_{2i+1}^T P^T_hi  (PE)
  O' slots DMA'd straight from PSUM to DRAM; host combines + normalizes.
"""

import os
import sys

import numpy as np

for _p in ("/opt/trn_rl_repo",):
    if _p not in sys.path and os.path.isdir(_p):
        sys.path.insert(0, _p)

import ml_dtypes

B, S, DM, DH = 4, 4096, 512, 64
NCORES = 8
NSLOTS = 8                    # query slots of 512 rows (full batch per core)
QG = 512
NKC_LOC = 16                  # local key chunks of 128 keys per core

BF16 = ml_dtypes.bfloat16

_PROGRAM = None


def _build_program():
    import concourse.bacc as bacc
    import concourse.mybir as mybir
    import concourse.tile as tile

    f32 = mybir.dt.float32
    bf16 = mybir.dt.bfloat16
    fp8 = mybir.dt.float8e4
    Exp = mybir.ActivationFunctionType.Exp
    DoubleRow = mybir.MatmulPerfMode.DoubleRow

    nc = bacc.Bacc("TRN2", target_bir_lowering=False, debug=False,
                   num_devices=NCORES)

    qT = nc.dram_tensor("qT", [DM, S], bf16, kind="ExternalInput").ap()
    kT = nc.dram_tensor("kT", [DM, S // 2], bf16, kind="ExternalInput").ap()
    vT = nc.dram_tensor("vT", [DM, S // 2], bf16, kind="ExternalInput").ap()
    # wq | wk | wv concatenated, pre-arranged on host to the SBUF layout
    # [128, (dm_chunk, 3*dh)] so the DMA has >=512B contiguous runs
    w3 = nc.dram_tensor("w3", [128, 4 * 3 * DH], bf16, kind="ExternalInput").ap()
    # thr[x, t] = 128*(2t + parity) + x : q-column threshold for the two
    # masked chunks of every slot's final pair
    thr = nc.dram_tensor("thr", [128, 2], f32, kind="ExternalInput").ap()
    # out[d, g, q] : rows 0..63 = unnormalized O'^T, row 64 = partial denom
    out = nc.dram_tensor("out", [DH + 1, NSLOTS, QG], f32,
                         kind="ExternalOutput").ap()

    qT_r = qT.rearrange("(c p) n -> p c n", p=128)
    kT_r = kT.rearrange("(c p) n -> p c n", p=128)
    vT_r = vT.rearrange("(c p) n -> p c n", p=128)

    with tile.TileContext(nc) as tc:
        with (
            tc.tile_pool(name="consts", bufs=1) as consts,
            tc.tile_pool(name="big", bufs=1) as big,
            tc.tile_pool(name="work", bufs=3) as work,
            tc.tile_pool(name="pst", bufs=2, space="PSUM") as pst,
            tc.tile_pool(name="pacc", bufs=2, space="PSUM") as pacc,
            tc.tile_pool(name="psm", bufs=2, space="PSUM") as psm,
        ):
            # ---- constants ----
            w_sb = consts.tile([128, 4, 3 * DH], bf16)
            nc.sync.dma_start(out=w_sb,
                              in_=w3.rearrange("p (c d) -> p c d", c=4))

            def wq_sb(c):
                return w_sb[:, c, 0:DH]

            def wk_sb(c):
                return w_sb[:, c, DH:2 * DH]

            def wv_sb(c):
                return w_sb[:, c, 2 * DH:3 * DH]

            thr_sb = consts.tile([128, 2], f32)
            yiota = consts.tile([128, QG], f32)
            masks = consts.tile([128, 2 * QG], bf16)

            def gen_masks():
                # on gpsimd: keeps the DVE queue free for projection copies.
                # Emitted after the fp8 zero memsets (Pool queue is in-order
                # and the is_ge waits on the thr DMA).
                nc.gpsimd.iota(yiota, pattern=[[1, QG]], base=0,
                               channel_multiplier=0,
                               allow_small_or_imprecise_dtypes=True)
                for t in range(2):
                    nc.gpsimd.tensor_scalar(
                        out=masks[:, t * QG:(t + 1) * QG], in0=yiota,
                        scalar1=thr_sb[:, t:t + 1], scalar2=None,
                        op0=mybir.AluOpType.is_ge)

            # ---- staged inputs / projected tensors ----
            qT_sb = big.tile([128, 4, S], bf16)
            kT_sb = big.tile([128, 4, S // 2], bf16)
            vT_sb = big.tile([128, 4, S // 2], bf16)
            # Q/K projected then quantized to fp8 for DoubleRow score matmuls;
            # layout [64, 2, n]: half 0 = real 64-dim values, half 1 = zeros
            # (DoubleRow contracts over both halves; the zero half is free and
            # lets the fp8 conversion be a single 64-partition copy)
            q8_sb = big.tile([64, 2, S], fp8)
            k8_sb = big.tile([64, 2, S // 2], fp8)
            # zero halves in pieces, first-needed first, so early score
            # matmuls aren't gated on one huge memset
            nc.gpsimd.memset(k8_sb[:, 1, 0:QG], 0.0)
            nc.gpsimd.memset(q8_sb[:, 1, 0:2 * QG], 0.0)
            gen_masks()
            for kp in range(1, 4):
                nc.gpsimd.memset(k8_sb[:, 1, kp * QG:(kp + 1) * QG], 0.0)
            for gp in range(1, 4):
                nc.gpsimd.memset(
                    q8_sb[:, 1, 2 * gp * QG:2 * (gp + 1) * QG], 0.0)
            v_sb = big.tile([128, NKC_LOC, DH + 1], bf16)
            nc.vector.memset(v_sb[:, :, DH:DH + 1], 1.0)
            out_sb = big.tile([DH + 1, NSLOTS, QG], f32)

            def dma_q(g):
                nc.sync.dma_start(out=qT_sb[:, :, g * QG:(g + 1) * QG],
                                  in_=qT_r[:, :, g * QG:(g + 1) * QG])

            def dma_k(c0, n):
                sl = slice(c0, c0 + n)
                nc.sync.dma_start(out=kT_sb[:, :, sl], in_=kT_r[:, :, sl])

            def dma_v(c0, n):
                sl = slice(c0, c0 + n)
                nc.sync.dma_start(out=vT_sb[:, :, sl], in_=vT_r[:, :, sl])

            def proj_q(g):
                sl = slice(g * QG, (g + 1) * QG)
                pp = psm.tile([64, QG], f32, tag="sm", name="pq")
                for c in range(4):
                    nc.tensor.matmul(pp, lhsT=wq_sb(c),
                                     rhs=qT_sb[:, c, sl],
                                     start=(c == 0), stop=(c == 3))
                nc.vector.tensor_copy(q8_sb[:, 0, sl], pp)

            def proj_k(col0, ncols, on_act=False):
                sl = slice(col0, col0 + ncols)
                pp = psm.tile([64, QG], f32, tag="sm", name="pk")
                for c in range(4):
                    nc.tensor.matmul(pp[:, 0:ncols], lhsT=wk_sb(c),
                                     rhs=kT_sb[:, c, sl],
                                     start=(c == 0), stop=(c == 3))
                if on_act:  # head only: ACT is idle before the first exp
                    nc.scalar.copy(k8_sb[:, 0, sl], pp[:, 0:ncols])
                else:
                    nc.vector.tensor_copy(k8_sb[:, 0, sl], pp[:, 0:ncols])

            def proj_v(kb):
                pv = psm.tile([128, DH], f32, tag="sm", name="pv")
                for c in range(4):
                    nc.tensor.matmul(pv,
                                     lhsT=vT_sb[:, c, kb * 128:(kb + 1) * 128],
                                     rhs=wv_sb(c),
                                     start=(c == 0), stop=(c == 3))
                nc.vector.tensor_copy(v_sb[:, kb, 0:DH], pv)

            # ---- software-pipelined main sequence ----
            # pairs[n] = (slot g, chunk-pair j, first, last, masked); stage A
            # (scores) is emitted one pair ahead of stage EP (exp + PV
            # accumulate) so the PE has score work queued while ACT runs the
            # previous pair's exp.  The last slot processes its masked
            # (diagonal) pair FIRST so the final tail chain skips the mask.
            pairs = []
            for g in range(NSLOTS):
                order = [g] + list(range(g)) if g == NSLOTS - 1 \
                    else list(range(g + 1))
                for idx, j in enumerate(order):
                    pairs.append((g, j, idx == 0, idx == len(order) - 1,
                                  j == g))
            npairs = len(pairs)

            sts = {}
            ots = {}

            def stage_A(n):
                g, jp, _, _, _ = pairs[n]
                st = pst.tile([128, 2 * QG], f32, tag="st", name="st")
                sts[n] = st
                qcols = q8_sb[:, :, g * QG:(g + 1) * QG]
                for h in range(2):
                    j = 2 * jp + h
                    nc.tensor.matmul(st[:, h * QG:(h + 1) * QG],
                                     lhsT=k8_sb[:, :, j * 128:(j + 1) * 128],
                                     rhs=qcols, start=True, stop=True,
                                     perf_mode=DoubleRow)

            def stage_EP(n):
                g, jp, first, last, masked = pairs[n]
                st = sts.pop(n)
                pt = work.tile([128, 2 * QG], bf16, tag="pt", name="pt")
                nc.scalar.activation(pt, st, Exp, scale=0.125)
                if masked:
                    nc.vector.tensor_mul(pt, pt, masks)
                ot = ots[g]
                for h in range(2):
                    j = 2 * jp + h
                    nc.tensor.matmul(ot, lhsT=v_sb[:, j, :],
                                     rhs=pt[:, h * QG:(h + 1) * QG],
                                     start=(first and h == 0),
                                     stop=(last and h == 1))
                if last:
                    # free the PSUM accumulator promptly, then DMA from SBUF
                    nc.vector.tensor_copy(out_sb[:, g, :], ot)
                    nc.sync.dma_start(out=out[:, g, :], in_=out_sb[:, g, :])
                    del ots[g]

            # input DMAs issued up-front, ordered by when the data is first
            # needed; the big SBUF tiles are persistent so transfers stream
            # in while compute chases the data.
            H = QG // 2
            dma_q(0)
            dma_k(0, H)
            dma_q(1)
            dma_k(H, H)
            dma_v(0, H)
            nc.sync.dma_start(out=thr_sb, in_=thr)
            dma_q(2)
            dma_v(H, H)
            dma_k(QG, QG)
            dma_v(QG, QG)
            dma_q(3)
            dma_q(4)
            dma_k(2 * QG, QG)
            dma_v(2 * QG, QG)
            dma_q(5)
            dma_q(6)
            dma_k(3 * QG, QG)
            dma_v(3 * QG, QG)
            dma_q(7)

            # deferred K/V projection work, keyed by the pair-iteration just
            # before the data's first consumer (in-order engine queues: work
            # emitted too early blocks ready work queued behind it).
            def T(m):
                return m * (m + 1) // 2

            work_at = {}

            def at(n, fn, *a):
                work_at.setdefault(max(0, n), []).append((fn, a))

            # Analytic DMA-arrival model (ns): transfers serialize on the DMA
            # resource in issue order; consumers see data ~900ns (sem prop)
            # after transfer end.  Used to pin DMA-gated projections via
            # tile_wait_until so the scheduling pass cannot hoist them ahead
            # of ready work (their Ldweights only depends on the weights and
            # would otherwise head-block the in-order PE queue).
            _t = 2000.0
            vis = {}
            for nm, nbytes in [("w", 196608), ("q0", 524288), ("k0a", 262144),
                               ("q1", 524288), ("k0b", 262144),
                               ("v0a", 262144), ("thr", 1024),
                               ("q2", 524288), ("v0b", 262144),
                               ("k1", 524288), ("v1", 524288),
                               ("q3", 524288), ("q4", 524288),
                               ("k2", 524288), ("v2", 524288),
                               ("q5", 524288), ("q6", 524288),
                               ("k3", 524288), ("v3", 524288),
                               ("q7", 524288)]:
                _t += max(nbytes / 360.0, 7.0)
                vis[nm] = _t + 900.0

            def wait_for(nm):
                return tc.tile_wait_until((vis[nm] - 300.0) * 1e-6)

            def prologue(g):
                """Q projection + accumulator for slot g."""
                with wait_for(f"q{g}"):
                    proj_q(g)
                ots[g] = pacc.tile([DH + 1, QG], f32, tag="acc", name="ot")

            # Deadlines (pair index whose EP precedes the emission).  Chosen
            # so same-engine priority order matches data-arrival order: a
            # Q/K projection emitted before its DMA lands gets its (always
            # ready) Ldweights popped by the idle PE and the matmul behind
            # it head-blocks the engine queue.
            def proj_k_w(nm, *a):
                with wait_for(nm):
                    proj_k(*a)

            def proj_v_w(blk):
                with wait_for(f"v{blk // 4}" if blk >= 4
                              else ("v0a" if blk < 2 else "v0b")):
                    proj_v(blk)

            for g2, nn in {2: 1, 3: 4, 4: 5, 5: 8, 6: 9, 7: 13}.items():
                at(nn, prologue, g2)
            at(3, proj_k_w, "k1", QG, QG)
            at(7, proj_k_w, "k2", 2 * QG, QG)
            at(11, proj_k_w, "k3", 3 * QG, QG)
            for blk in range(4, 4 * 4):
                at(T(blk // 2) + blk // 2 - 2, proj_v_w, blk)

            # head: minimal chain to the first exp
            with wait_for("q0"):
                proj_q(0)
            with wait_for("k0a"):
                proj_k(0, QG // 2, on_act=True)
            ots[0] = pacc.tile([DH + 1, QG], f32, tag="acc", name="ot")
            stage_A(0)
            with wait_for("k0b"):
                proj_k(QG // 2, QG // 2)
            proj_v_w(0)
            proj_v_w(1)
            at(0, proj_v_w, 2)
            at(0, proj_v_w, 3)
            prologue(1)
            for n in range(npairs):
                if n + 1 < npairs:
                    stage_A(n + 1)
                stage_EP(n)
                for fn, a in work_at.get(n, ()):
                    fn(*a)

    nc.compile()
    return nc


def _get_program():
    global _PROGRAM
    if _PROGRAM is None:
        _PROGRAM = _build_program()
    return _PROGRAM


def _thr_table(p):
    t = np.empty((128, 2), np.float32)
    x = np.arange(128, dtype=np.float32)
    t[:, 0] = 128 * p + x
    t[:, 1] = 128 * (2 + p) + x
    return t


def kernel(q, k, v, Wq, Wk, Wv, mask):
    assert q.shape == (B, S, DM) and k.shape == (B, S, DM)
    assert v.shape == (B, S, DM) and Wq.shape == (DM, DH)

    nc = _get_program()
    from concourse.bass_utils import run_bass_kernel_spmd

    q = np.asarray(q, np.float32)
    k = np.asarray(k, np.float32)
    v = np.asarray(v, np.float32)
    w3 = np.concatenate([np.asarray(Wq, np.float32),
                         np.asarray(Wk, np.float32),
                         np.asarray(Wv, np.float32)], axis=1).astype(BF16)
    # pre-arrange to the SBUF layout [128 partitions, (dm_chunk, 3*dh)]
    w3 = np.ascontiguousarray(
        w3.reshape(4, 128, 3 * DH).transpose(1, 0, 2).reshape(128, 12 * DH))

    qT_b = [np.ascontiguousarray(q[b].T).astype(BF16) for b in range(B)]

    in_maps = []
    for c in range(NCORES):
        b, p = c % 4, c // 4
        k_sel = k[b].reshape(S // 128, 128, DM)[p::2].reshape(S // 2, DM)
        v_sel = v[b].reshape(S // 128, 128, DM)[p::2].reshape(S // 2, DM)
        in_maps.append({
            "qT": qT_b[b],
            "kT": np.ascontiguousarray(k_sel.T).astype(BF16),
            "vT": np.ascontiguousarray(v_sel.T).astype(BF16),
            "w3": w3,
            "thr": _thr_table(p),
        })

    res = run_bass_kernel_spmd(nc, in_maps, core_ids=list(range(NCORES)))

    O = np.empty((B, S, DH), np.float32)
    for b in range(B):
        tot = res.results[b]["out"] + res.results[b + 4]["out"]
        denom = tot[DH]                       # [8, 512]
        o = tot[0:DH].transpose(1, 2, 0)      # [8, 512, 64]
        O[b] = (o / denom[:, :, None]).reshape(S, DH)
    return O
